# revision 12
# baseline (speedup 1.0000x reference)
"""Trainium2 Bass kernel for nn_CreateOverlappingWindows.

out[b, t, w*C + c] = x_padded[b, t + w, c]  (SAME zero padding, n_context=9)

Flattening (w, c) -> 494 contiguous values, each output row is a contiguous
494-element window of the zero-padded flattened input:
    out[b, t, :] = xpad_flat[b, t*C : t*C + W*C]

Strategy (memory-regime, bf16 end-to-end):
  * All 4 per-core batches go through SBUF.  128 partitions x 16 rows per
    batch (T padded to 2048 on device, trimmed on host).
  * SBUF AXI ports are the binding resource: port = ((p>>2)&7)<<1|(p>>6),
    27 GB/s each.  A HWDGE InstDMACopy splits its outer dim over SDMA
    engines in contiguous runs (engines = largest divisor <= 16), so a
    64-partition DMA gives each engine exactly one 4-partition port
    group.  Batches 0-2 store as group A (partitions 0-63, even ports)
    on the sync ring CONCURRENT with group B (partitions 64-127, odd
    ports) on the scalar ring - disjoint port halves.  Batch 3 stores on
    the gpsimd SWDGE ring (its descriptor spray is natively
    port-aligned across all 16 engines), a third concurrent stream.
  * Loads also run on the gpsimd ring: keeps the HWDGE FIFOs free of
    512 small descriptors (~12.5ns/desc ring dispatch) so stores start
    as soon as the first expansion chunk lands.
  * The 26 -> 494 window expansion runs per chunk of (4, 6, 6) rows:
    the 4-row chunk is DVE-only so the store pipe starts early; the
    6-row chunks split DVE (int32-viewed copies, 1-port mode so SWDGE
    descriptor generation is never locked out) / ACT (native bf16 -
    ACT's fp path would round int32 views).  A dummy ACT copy on a
    scratch tile preloads the activation table off the critical path.
  * Engine-program order does NOT order a dma_start after an in-flight
    copy: every store is gated on the ev/ea semaphores (true completion).

Sharding: pure data parallel - batch 32 split 4-per-core across 8 cores.
"""

import sys

sys.path.insert(0, "/opt/trn_rl_repo")

import ml_dtypes
import numpy as np
from concourse import bass, mybir
from concourse.ap import AP
from concourse.bass_utils import run_bass_kernel_spmd

_BF16 = mybir.dt.bfloat16
_I32 = mybir.dt.int32
_NPBF16 = ml_dtypes.bfloat16

_NCORES = 8
_B, _T, _C = 32, 2000, 26
_NCTX = 9
_W = 2 * _NCTX + 1  # 19
_WC = _W * _C  # 494
_PAD = _NCTX * _C  # 234
_BPC = _B // _NCORES  # 4 batches per core

_P = 128  # partitions per batch
_R = 16  # output rows per partition
_TV = _P * _R  # 2048 device-side rows (rows 2000+ are discarded on host)
_SEG = _R * _C + (_WC - _C)  # 884: input slice length incl. halo
_NP = (_P - 1) * _R * _C + _SEG  # 53716 padded flat input length per batch
_RW = _R * _WC  # 7904 output elems per partition per batch
_TWC = _TV * _WC  # 1011712 device-side output elems per batch
_FI = _BPC * _SEG  # 3536 free elems/partition, input tile
_FO = _BPC * _RW  # 31616 free elems/partition, output tile

# per batch: 3 chunks of (4, 6, 6) rows; DVE rows per chunk, rest ACT
_CR = (4, 6, 6)
_CR0 = (0, 4, 10)  # start row of each chunk
_DR = (4, 3, 3)  # DVE rows (ACT gets _CR - _DR: 0, 3, 3)
_NCH = len(_CR)
_HG = _P // 2  # 64 partitions per store group
_GB = _BPC - 1  # batch stored via the gpsimd (SWDGE) ring

_nc_cache = None


def _build():
    global _nc_cache
    if _nc_cache is not None:
        return _nc_cache
    nc = bass.Bass()
    xp = nc.declare_dram_parameter("xp", [_BPC, _NP], _BF16, isOutput=False)
    out = nc.declare_dram_parameter("out", [_BPC, _TV, _WC], _BF16, isOutput=True)

    with (
        nc.sbuf_tensor([128, _FI], _BF16) as tin,
        nc.sbuf_tensor([128, _FO], _BF16) as tout,
        nc.sbuf_tensor([128, 2], _BF16) as scratch,
        nc.Block() as block,
        nc.semaphore("l0") as l0,
        nc.semaphore("l1") as l1,
        nc.semaphore("l2") as l2,
        nc.semaphore("l3") as l3,
        nc.semaphore("ev") as ev,
        nc.semaphore("ea") as ea,
        nc.semaphore("ss") as ss,
    ):
        lsem = [l0, l1, l2, l3]

        def store_chunk(e, b, j, g):
            """Store chunk j of batch b; g=0/1: 64-partition half, g=2: all 128."""
            r0, n = _CR0[j], _CR[j] * _WC
            np_, g = (_P, 0) if g == 2 else (_HG, g)
            return e.dma_start(
                out=AP(
                    out,
                    b * _TWC + g * _HG * _RW + r0 * _WC,
                    [[_RW, np_], [1, n]],
                ),
                in_=AP(
                    tout,
                    g * _HG * _FO + b * _RW + r0 * _WC,
                    [[_FO, np_], [1, n]],
                ),
            ).then_inc(ss, 16)

        def wait_chunk(e, b, j):
            e.wait_ge(ev, _NCH * b + j + 1)
            if j > 0:  # chunk 0 has no ACT rows
                e.wait_ge(ea, 2 * b + j)

        @block.gpsimd
        def _(e):
            for b in range(_BPC):
                e.dma_start(
                    out=AP(tin, b * _SEG, [[_FI, _P], [1, _SEG]]),
                    in_=AP(xp, b * _NP, [[_R * _C, _P], [1, _SEG]]),
                ).then_inc(lsem[b], 16)
            for j in range(_NCH):
                wait_chunk(e, _GB, j)
                store_chunk(e, _GB, j, 2)

        @block.sync
        def _(e):
            for b in range(_BPC - 1):
                for j in range(_NCH):
                    wait_chunk(e, b, j)
                    store_chunk(e, b, j, 0)  # group A: even ports
            e.wait_ge(ss, 16 * (2 * _NCH * (_BPC - 1) + _NCH))

        @block.vector
        def _(v):
            for b in range(_BPC):
                v.wait_ge(lsem[b], 16)
                for j in range(_NCH):
                    r0 = _CR0[j]
                    v.tensor_copy(
                        out=AP(
                            tout,
                            b * _RW + r0 * _WC,
                            [[_FO, _P], [_WC, _DR[j]], [1, _WC]],
                        ).bitcast(_I32),
                        in_=AP(
                            tin,
                            b * _SEG + r0 * _C,
                            [[_FI, _P], [_C, _DR[j]], [1, _WC]],
                        ).bitcast(_I32),
                    ).then_inc(ev, 1)

        @block.scalar
        def _(e):
            # dummy ACT op: pulls the activation table load off the
            # critical path (first real copy would otherwise pay ~1.3us)
            e.copy(out=AP(scratch, 0, [[2, 1], [1, 2]]),
                   in_=AP(scratch, 0, [[2, 1], [1, 2]]))
            for b in range(_BPC):
                e.wait_ge(ev, _NCH * b + 1)
                if b < _BPC - 1:
                    store_chunk(e, b, 0, 1)  # chunk 0, group B: DVE-only rows
                e.wait_ge(lsem[b], 16)
                for j in range(1, _NCH):
                    r0 = _CR0[j] + _DR[j]
                    nr = _CR[j] - _DR[j]
                    e.copy(
                        out=AP(
                            tout,
                            b * _RW + r0 * _WC,
                            [[_FO, _P], [_WC, nr], [1, _WC]],
                        ),
                        in_=AP(
                            tin,
                            b * _SEG + r0 * _C,
                            [[_FI, _P], [_C, nr], [1, _WC]],
                        ),
                    ).then_inc(ea, 1)
                if b < _BPC - 1:
                    for j in range(1, _NCH):
                        wait_chunk(e, b, j)
                        store_chunk(e, b, j, 1)

    _nc_cache = nc
    return nc


def _make_in_maps(x: np.ndarray) -> list[dict]:
    """x: [B, T, C] float32 -> per-core padded bf16 flat inputs."""
    xb = np.asarray(x, dtype=np.float32).astype(_NPBF16)
    xpad = np.zeros((_B, _NP), _NPBF16)
    xpad[:, _PAD : _PAD + _T * _C] = xb.reshape(_B, _T * _C)
    return [
        {"xp": np.ascontiguousarray(xpad[i * _BPC : (i + 1) * _BPC])}
        for i in range(_NCORES)
    ]


def _gather_out(results) -> np.ndarray:
    return np.concatenate(
        [np.asarray(r["out"]).astype(np.float32)[:, :_T, :] for r in results],
        axis=0,
    ).reshape(_B, _T, _WC)


def kernel(x: np.ndarray) -> np.ndarray:
    assert np.asarray(x).shape == (_B, _T, _C)
    nc = _build()
    res = run_bass_kernel_spmd(nc, _make_in_maps(x), list(range(_NCORES)))
    return _gather_out(res.results)


# revision 13
# speedup vs baseline: 1.0541x; 1.0541x over previous
"""Trainium2 Bass kernel for nn_CreateOverlappingWindows.

out[b, t, w*C + c] = x_padded[b, t + w, c]  (SAME zero padding, n_context=9)

Flattening (w, c) -> 494 contiguous values, each output row is a contiguous
494-element window of the zero-padded flattened input:
    out[b, t, :] = xpad_flat[b, t*C : t*C + W*C]

Strategy (memory-regime, bf16 end-to-end):
  * All 4 per-core batches go through SBUF.  128 partitions x 16 rows per
    batch (T padded to 2048 on device, trimmed on host).
  * SBUF AXI ports are the binding resource: port = ((p>>2)&7)<<1|(p>>6),
    27 GB/s each.  A HWDGE InstDMACopy splits its outer dim over SDMA
    engines in contiguous runs (engines = largest divisor <= 16), so a
    64-partition DMA gives each engine exactly one 4-partition port
    group.  Batches 0-2 store as group A (partitions 0-63, even ports)
    on the sync ring CONCURRENT with group B (partitions 64-127, odd
    ports) on the scalar ring - disjoint port halves.  Batch 3 is
    expanded by DVE alone and stored on the gpsimd SWDGE ring (natively
    port-aligned spray), a third stream that overlaps the ring stores.
  * Batch 0 loads via the two HWDGE rings (one 64-partition half each,
    both bumping the batch-0 semaphore +16 -> wait 32) for the earliest
    possible pipeline start; batches 1-3 load via gpsimd, keeping the
    HWDGE FIFOs free of small descriptors once stores begin.
  * The 26 -> 494 window expansion runs per chunk of (4, 6, 6) rows:
    the 4-row chunk is DVE-only so the store pipe starts early; the
    6-row chunks of batches 0-2 split DVE (int32-viewed copies, 1-port
    mode so SWDGE generation is never locked out) / ACT (native bf16 -
    ACT's fp path would round int32 views).  A dummy ACT copy on a
    scratch tile preloads the activation table off the critical path.
  * Engine-program order does NOT order a dma_start after an in-flight
    copy: every store is gated on the ev/ea semaphores (true completion).

Sharding: pure data parallel - batch 32 split 4-per-core across 8 cores.
"""

import sys

sys.path.insert(0, "/opt/trn_rl_repo")

import ml_dtypes
import numpy as np
from concourse import bass, mybir
from concourse.ap import AP
from concourse.bass_utils import run_bass_kernel_spmd

_BF16 = mybir.dt.bfloat16
_I32 = mybir.dt.int32
_NPBF16 = ml_dtypes.bfloat16

_NCORES = 8
_B, _T, _C = 32, 2000, 26
_NCTX = 9
_W = 2 * _NCTX + 1  # 19
_WC = _W * _C  # 494
_PAD = _NCTX * _C  # 234
_BPC = _B // _NCORES  # 4 batches per core

_P = 128  # partitions per batch
_R = 16  # output rows per partition
_TV = _P * _R  # 2048 device-side rows (rows 2000+ are discarded on host)
_SEG = _R * _C + (_WC - _C)  # 884: input slice length incl. halo
_NP = (_P - 1) * _R * _C + _SEG  # 53716 padded flat input length per batch
_RW = _R * _WC  # 7904 output elems per partition per batch
_TWC = _TV * _WC  # 1011712 device-side output elems per batch
_FI = _BPC * _SEG  # 3536 free elems/partition, input tile
_FO = _BPC * _RW  # 31616 free elems/partition, output tile

# per batch: 3 chunks of (4, 6, 6) rows; DVE rows per chunk, rest ACT.
# Batch _GB is expanded entirely by DVE and stored via the gpsimd ring.
_CR = (4, 6, 6)
_CR0 = (0, 4, 10)  # start row of each chunk
_DR = (4, 3, 3)  # DVE rows for batches 0.._GB-1 (ACT gets the rest)
_NCH = len(_CR)
_HG = _P // 2  # 64 partitions per store group
_GB = _BPC - 1  # batch stored via the gpsimd (SWDGE) ring

_nc_cache = None


def _build():
    global _nc_cache
    if _nc_cache is not None:
        return _nc_cache
    nc = bass.Bass()
    xp = nc.declare_dram_parameter("xp", [_BPC, _NP], _BF16, isOutput=False)
    out = nc.declare_dram_parameter("out", [_BPC, _TV, _WC], _BF16, isOutput=True)

    with (
        nc.sbuf_tensor([128, _FI], _BF16) as tin,
        nc.sbuf_tensor([128, _FO], _BF16) as tout,
        nc.sbuf_tensor([128, 2], _BF16) as scratch,
        nc.Block() as block,
        nc.semaphore("l0") as l0,
        nc.semaphore("l1") as l1,
        nc.semaphore("l2") as l2,
        nc.semaphore("l3") as l3,
        nc.semaphore("ev") as ev,
        nc.semaphore("ea") as ea,
        nc.semaphore("ss") as ss,
    ):
        lsem = [l0, l1, l2, l3]

        def load_b0_half(e, g):
            return e.dma_start(
                out=AP(tin, g * _HG * _FI, [[_FI, _HG], [1, _SEG]]),
                in_=AP(xp, g * _HG * _R * _C, [[_R * _C, _HG], [1, _SEG]]),
            ).then_inc(lsem[0], 16)

        def store_chunk(e, b, j, g):
            """Store chunk j of batch b; g=0/1: 64-partition half, g=2: all 128."""
            r0, n = _CR0[j], _CR[j] * _WC
            np_, g = (_P, 0) if g == 2 else (_HG, g)
            return e.dma_start(
                out=AP(
                    out,
                    b * _TWC + g * _HG * _RW + r0 * _WC,
                    [[_RW, np_], [1, n]],
                ),
                in_=AP(
                    tout,
                    g * _HG * _FO + b * _RW + r0 * _WC,
                    [[_FO, np_], [1, n]],
                ),
            ).then_inc(ss, 16)

        def wait_chunk(e, b, j):
            e.wait_ge(ev, _NCH * b + j + 1)
            if j > 0 and b != _GB:  # chunk 0 / gpsimd batch: no ACT rows
                e.wait_ge(ea, 2 * b + j)

        def expand(eng, b, r0, nr, as_i32):
            o = AP(tout, b * _RW + r0 * _WC, [[_FO, _P], [_WC, nr], [1, _WC]])
            i = AP(tin, b * _SEG + r0 * _C, [[_FI, _P], [_C, nr], [1, _WC]])
            if as_i32:
                return eng.tensor_copy(out=o.bitcast(_I32), in_=i.bitcast(_I32))
            return eng.copy(out=o, in_=i)

        @block.gpsimd
        def _(e):
            for b in range(1, _BPC):
                e.dma_start(
                    out=AP(tin, b * _SEG, [[_FI, _P], [1, _SEG]]),
                    in_=AP(xp, b * _NP, [[_R * _C, _P], [1, _SEG]]),
                ).then_inc(lsem[b], 16)
            for j in range(_NCH):
                wait_chunk(e, _GB, j)
                store_chunk(e, _GB, j, 2)

        @block.sync
        def _(e):
            load_b0_half(e, 0)
            for b in range(_BPC - 1):
                for j in range(_NCH):
                    wait_chunk(e, b, j)
                    store_chunk(e, b, j, 0)  # group A: even ports
            e.wait_ge(ss, 16 * (2 * _NCH * (_BPC - 1) + _NCH))

        @block.vector
        def _(v):
            for b in range(_BPC):
                v.wait_ge(lsem[b], 32 if b == 0 else 16)
                for j in range(_NCH):
                    nr = _CR[j] if b == _GB else _DR[j]
                    expand(v, b, _CR0[j], nr, True).then_inc(ev, 1)

        @block.scalar
        def _(e):
            load_b0_half(e, 1)
            # dummy ACT op: pulls the activation table load off the
            # critical path (first real copy would otherwise pay ~1.3us)
            e.copy(out=AP(scratch, 0, [[2, 1], [1, 2]]),
                   in_=AP(scratch, 0, [[2, 1], [1, 2]]))
            for b in range(_BPC - 1):
                e.wait_ge(ev, _NCH * b + 1)
                store_chunk(e, b, 0, 1)  # chunk 0, group B: DVE-only rows
                e.wait_ge(lsem[b], 32 if b == 0 else 16)
                for j in range(1, _NCH):
                    expand(
                        e, b, _CR0[j] + _DR[j], _CR[j] - _DR[j], False
                    ).then_inc(ea, 1)
                for j in range(1, _NCH):
                    wait_chunk(e, b, j)
                    store_chunk(e, b, j, 1)

    _nc_cache = nc
    return nc


def _make_in_maps(x: np.ndarray) -> list[dict]:
    """x: [B, T, C] float32 -> per-core padded bf16 flat inputs."""
    xb = np.asarray(x, dtype=np.float32).astype(_NPBF16)
    xpad = np.zeros((_B, _NP), _NPBF16)
    xpad[:, _PAD : _PAD + _T * _C] = xb.reshape(_B, _T * _C)
    return [
        {"xp": np.ascontiguousarray(xpad[i * _BPC : (i + 1) * _BPC])}
        for i in range(_NCORES)
    ]


def _gather_out(results) -> np.ndarray:
    return np.concatenate(
        [np.asarray(r["out"]).astype(np.float32)[:, :_T, :] for r in results],
        axis=0,
    ).reshape(_B, _T, _WC)


def kernel(x: np.ndarray) -> np.ndarray:
    assert np.asarray(x).shape == (_B, _T, _C)
    nc = _build()
    res = run_bass_kernel_spmd(nc, _make_in_maps(x), list(range(_NCORES)))
    return _gather_out(res.results)


# revision 14
# speedup vs baseline: 1.1423x; 1.0837x over previous
"""Trainium2 Bass kernel for nn_CreateOverlappingWindows.

out[b, t, w*C + c] = x_padded[b, t + w, c]  (SAME zero padding, n_context=9)

Flattening (w, c) -> 494 contiguous values, each output row is a contiguous
494-element window of the zero-padded flattened input:
    out[b, t, :] = xpad_flat[b, t*C : t*C + W*C]

Key identity: rows of equal phase j = t mod 19 are CONTIGUOUS in xpad:
    out[b, j::19, :].flat == xpad[b, 26j : 26j + 494*K]   (494 = 19*26)
so with a phase-major device output layout (host de-interleaves), the
entire 19x window expansion is pure DMA descriptor geometry - no
compute engines at all.

Strategy (memory-regime, bf16 end-to-end):
  * Each batch's padded input (52,836 elems) is split into 16 pieces of
    3273 elems (+468 halo -> 3741 per piece, 1.13x read amplification),
    placed in partitions {32 + 4i}: those map 1:1 onto all 16 SBUF AXI
    ports (port = ((p>>2)&7)<<1|(p>>6)), so every store DMA runs all 16
    SDMA engines with zero port sharing.
  * Store DMA per batch: 3-level AP, 16 partitions x 19 phases x 3273
    contiguous elems (6546-B descriptors).  DRAM side is phase-major
    [19, 52368]; host reshapes/interleaves (free) and trims the <=1 pad
    row per phase.
  * Loads: 16 descs per batch (7482 B each).  Two per HWDGE ring, then
    each ring stores two batches, gated per-batch on its load semaphore.
  * Stores drain at the HBM write roofline; DVE/ACT/gpsimd stay idle.

Sharding: pure data parallel - batch 32 split 4-per-core across 8 cores.
"""

import sys

sys.path.insert(0, "/opt/trn_rl_repo")

import ml_dtypes
import numpy as np
from concourse import bass, mybir
from concourse.ap import AP
from concourse.bass_utils import run_bass_kernel_spmd

_BF16 = mybir.dt.bfloat16
_NPBF16 = ml_dtypes.bfloat16

_NCORES = 8
_B, _T, _C = 32, 2000, 26
_NCTX = 9
_W = 2 * _NCTX + 1  # 19
_WC = _W * _C  # 494
_PAD = _NCTX * _C  # 234
_BPC = _B // _NCORES  # 4 batches per core

_NPIECE = 16  # input pieces per batch = SDMA engines = AXI ports
_PL = 3273  # piece stride (elems); 16*3273 = 52368 covers the input
_HALO = _WC - _C  # 468
_PSEG = _PL + _HALO  # 3741 elems actually loaded per piece
_NP = _NPIECE * _PL + _HALO  # 52836 padded flat input length per batch
_K = (_T + _W - 1) // _W  # 106 rows per phase (ceil)
_PHL = _NPIECE * _PL  # 52368 elems stored per phase (= 494*106 + 4 pad)
_OB = _W * _PHL  # 994992 output elems per batch (phase-major)
_F2 = _BPC * _PSEG  # 14964 free elems/partition
_P0 = 32  # base partition: {32+4i} hit all 16 distinct AXI ports
_PSTEP = 4

_nc_cache = None


def _build():
    global _nc_cache
    if _nc_cache is not None:
        return _nc_cache
    nc = bass.Bass()
    xp = nc.declare_dram_parameter("xp", [_BPC, _NP], _BF16, isOutput=False)
    out = nc.declare_dram_parameter("out", [_BPC, _OB], _BF16, isOutput=True)

    with (
        nc.sbuf_tensor([128, _F2], _BF16) as tin,
        nc.Block() as block,
        nc.semaphore("l0") as l0,
        nc.semaphore("l1") as l1,
        nc.semaphore("l2") as l2,
        nc.semaphore("l3") as l3,
        nc.semaphore("ss") as ss,
    ):
        lsem = [l0, l1, l2, l3]

        def load_batch(e, b):
            return e.dma_start(
                out=AP(
                    tin,
                    _P0 * _F2 + b * _PSEG,
                    [[_PSTEP * _F2, _NPIECE], [1, _PSEG]],
                ),
                in_=AP(xp, b * _NP, [[_PL, _NPIECE], [1, _PSEG]]),
            ).then_inc(lsem[b], 16)

        def store_batch(e, b):
            return e.dma_start(
                out=AP(
                    out,
                    b * _OB,
                    [[_PL, _NPIECE], [_PHL, _W], [1, _PL]],
                ),
                in_=AP(
                    tin,
                    _P0 * _F2 + b * _PSEG,
                    [[_PSTEP * _F2, _NPIECE], [_C, _W], [1, _PL]],
                ),
            ).then_inc(ss, 16)

        @block.sync
        def _(e):
            load_batch(e, 0)
            load_batch(e, 2)
            e.wait_ge(lsem[0], 16)
            store_batch(e, 0)
            e.wait_ge(lsem[2], 16)
            store_batch(e, 2)
            e.wait_ge(ss, 16 * _BPC)

        @block.scalar
        def _(e):
            load_batch(e, 1)
            load_batch(e, 3)
            e.wait_ge(lsem[1], 16)
            store_batch(e, 1)
            e.wait_ge(lsem[3], 16)
            store_batch(e, 3)

    _nc_cache = nc
    return nc


def _make_in_maps(x: np.ndarray) -> list[dict]:
    """x: [B, T, C] float32 -> per-core padded bf16 flat inputs."""
    xb = np.asarray(x, dtype=np.float32).astype(_NPBF16)
    xpad = np.zeros((_B, _NP), _NPBF16)
    xpad[:, _PAD : _PAD + _T * _C] = xb.reshape(_B, _T * _C)
    return [
        {"xp": np.ascontiguousarray(xpad[i * _BPC : (i + 1) * _BPC])}
        for i in range(_NCORES)
    ]


def _gather_out(results) -> np.ndarray:
    full = np.empty((_B, _T, _WC), np.float32)
    for i, r in enumerate(results):
        dev = np.asarray(r["out"]).astype(np.float32)  # [BPC, W*PHL]
        dev = dev.reshape(_BPC, _W, _PHL)
        for j in range(_W):
            k = (_T - j + _W - 1) // _W  # rows of phase j (105 or 106)
            rows = dev[:, j, : k * _WC].reshape(_BPC, k, _WC)
            full[i * _BPC : (i + 1) * _BPC, j::_W, :] = rows
    return full


def kernel(x: np.ndarray) -> np.ndarray:
    assert np.asarray(x).shape == (_B, _T, _C)
    nc = _build()
    res = run_bass_kernel_spmd(nc, _make_in_maps(x), list(range(_NCORES)))
    return _gather_out(res.results)
